# revision 9
# baseline (speedup 1.0000x reference)
"""FFM cell kernel for Trainium2, 8 NeuronCores, batch-parallel.

Math: per batch element b,
    gated[t,m] = (x@W_pre + b_pre)[t,m] * sigmoid(x@W_gin + b_gin)[t,m]
    state[t,m,c] = sum_{s<=t} exp((a_m + i*b_c)*(t-s)) * gated[s,m]
    zm = [state.re, state.im] @ W_mix + b_mix
    out = LN(zm * sig(gout)) + skip * (1 - sig(gout))

The complex diagonal recurrence is decoupled into two *real* first-order
scans using angle addition (z is real):
    A[t,ch] = e^{a_m} A[t-1,ch] + cos(b_c t) z[t,m]
    B[t,ch] = e^{a_m} B[t-1,ch] + sin(b_c t) z[t,m]
    state_re = cos(b_c t) A + sin(b_c t) B
    state_im = sin(b_c t) A - cos(b_c t) B
mapped onto the DVE hardware scan (tensor_tensor_scan), channels (m,c) on
partitions, time on the free dim; cos/sin tables host-precomputed.

Engine split (latency-oriented):
  PE   : all matmuls, 2-deep software-pipelined so group g+2's broadcast
         (EXPM) matmul is issued before group g's mix matmuls.
  DVE  : modulation TTs, the two scans, the four demod products, LN stats.
  Pool : demod add/sub (sre/sim), sk2 = (1-gate)*skip, final out add.
  Act  : PSUM->SBUF bf16 copies, sigmoids (gate and 1-gate directly from
         PSUM via scale=-1), LN normalize, rsqrt.
Matmuls run as float32r/bf16. Sharding: batch element -> core; everything
replicated; no collectives.
"""

import numpy as np

B, T, D = 8, 1024, 512
TR, CTX, OUT = 64, 16, 512
EPS = 1e-6
NCH = TR * CTX   # 1024 scan channels
NG = NCH // 128  # 8 channel groups of 128 partitions
NT = T // 128    # 8 token tiles
KD = D // 128    # 4 contraction chunks over D
NTE = 6          # token tiles whose zm accumulates in PSUM during the loop

STREAM_BF16 = True   # bf16 modulation/post stream (DVE 2x) vs fp32

_cache = {}


def build_program(n_rep=1, with_bias=True):
    """Build + compile the Bass program (single NEFF, SPMD on 8 cores).

    n_rep > 1 repeats the whole pipeline (incl. DMA loads) for
    differential wall-clock timing; each repeat rewrites the output."""
    import concourse.bacc as bacc
    import concourse.tile as tile
    import concourse.mybir as mybir
    from concourse.alu_op_type import AluOpType as op

    f32 = mybir.dt.float32
    f32r = mybir.dt.float32r
    bf16 = mybir.dt.bfloat16
    sdt = bf16 if STREAM_BF16 else f32
    AF = mybir.ActivationFunctionType

    def r(ap):  # fp32 -> fp32r view for fast PE matmul
        return ap.bitcast(f32r) if ap.dtype == f32 else ap

    wb = with_bias
    nc = bacc.Bacc("TRN2", target_bir_lowering=False, debug=False)

    def din(name, shape, dt=f32):
        return nc.dram_tensor(name, shape, dt, kind="ExternalInput").ap()

    xT = din("xT", (D, T), sdt)
    Wpre = din("Wpre", (D, TR), sdt)
    Wgin = din("Wgin", (D, TR), sdt)
    Wgout = din("Wgout", (D, OUT), sdt)
    Wskip = din("Wskip", (D, OUT), sdt)
    Wmre = din("Wmre", (NCH, OUT), sdt)   # W_mix real rows, (m,c) order
    Wmim = din("Wmim", (NCH, OUT), sdt)
    COS = din("COS", (128, T), sdt)       # row rr: cos(b_{rr%16} * t)
    SIN = din("SIN", (128, T), sdt)
    DEC = din("DEC", (128, NG))           # col g: exp(-|a_{8g + rr//16}|)
    EXPM = din("EXPM", (TR, NCH), f32r)   # 0/1: EXPM[m, col] = (m == col//16)
    bpre = din("bpre", (TR, 1))
    bgin = din("bgin", (TR, 1))
    bgout = din("bgout", (1, OUT), f32r)
    bskip = din("bskip", (1, OUT), f32r)
    bmix = din("bmix", (1, OUT), f32r)
    ones = din("ones", (1, 128), f32r)
    out_d = nc.dram_tensor("out", (T, OUT), f32, kind="ExternalOutput").ap()

    outq = [nc.sync, nc.scalar, nc.gpsimd]

    with tile.TileContext(nc) as tc:
      for _rep in range(n_rep):
        with (
            tc.tile_pool(name="singles", bufs=1) as singles,
            tc.tile_pool(name="states", bufs=1) as states,
        ):
            def load(ap_dram, shape, tag, dt=f32, q=nc.sync):
                t = singles.tile(shape, dt, tag=tag, name=tag)
                q.dma_start(out=t, in_=ap_dram)
                return t

            ldq = [nc.sync, nc.scalar, nc.gpsimd, nc.scalar]
            xT_sb = [load(xT[k * 128:(k + 1) * 128, :], [128, T], f"xT{k}",
                          sdt, ldq[k]) for k in range(KD)]
            Wpre_sb = [load(Wpre[k * 128:(k + 1) * 128, :], [128, TR],
                            f"wpre{k}", sdt) for k in range(KD)]
            Wgin_sb = [load(Wgin[k * 128:(k + 1) * 128, :], [128, TR],
                            f"wgin{k}", sdt) for k in range(KD)]
            Wgout_sb = [load(Wgout[k * 128:(k + 1) * 128, :], [128, OUT],
                             f"wgout{k}", sdt, nc.scalar) for k in range(KD)]
            Wskip_sb = [load(Wskip[k * 128:(k + 1) * 128, :], [128, OUT],
                             f"wskip{k}", sdt, nc.scalar) for k in range(KD)]
            Wmre_sb = [load(Wmre[g * 128:(g + 1) * 128, :], [128, OUT],
                            f"wmre{g}", sdt, nc.gpsimd) for g in range(NG)]
            Wmim_sb = [load(Wmim[g * 128:(g + 1) * 128, :], [128, OUT],
                            f"wmim{g}", sdt, nc.gpsimd) for g in range(NG)]
            COS_sb = load(COS, [128, T], "cos", sdt)
            SIN_sb = load(SIN, [128, T], "sin", sdt)
            DEC_sb = load(DEC, [128, NG], "dec")
            EXPM_sb = load(EXPM, [TR, NCH], "expm", f32r)
            bpre_sb = load(bpre, [TR, 1], "bpre")
            bgin_sb = load(bgin, [TR, 1], "bgin")
            bgout_sb = load(bgout, [1, OUT], "bgout", f32r)
            bskip_sb = load(bskip, [1, OUT], "bskip", f32r)
            bmix_sb = load(bmix, [1, OUT], "bmix", f32r)

            ones_sb = load(ones, [1, 128], "ones", f32r)
            eps_sb = singles.tile([128, 1], f32, tag="eps")
            nc.vector.memset(eps_sb, EPS)

            sre = [states.tile([128, T], sdt, tag=f"sre{g}", name=f"sre{g}")
                   for g in range(NG)]
            sim = [states.tile([128, T], sdt, tag=f"sim{g}", name=f"sim{g}")
                   for g in range(NG)]
            gsigs = [states.tile([128, OUT], f32, tag=f"gsig{ti}",
                                 name=f"gsig{ti}") for ti in range(NT)]
            skips = [states.tile([128, OUT], f32, tag=f"skip{ti}",
                                 name=f"skip{ti}") for ti in range(NT)]
            omgs = [states.tile([128, OUT], f32, tag=f"omg{ti}",
                                name=f"omg{ti}") for ti in range(NT)]
            sk2s = [states.tile([128, OUT], f32, tag=f"sk2{ti}",
                                name=f"sk2{ti}") for ti in range(NT)]

            # ---- stage A: gated = (pre + bpre) * sig(gin + bgin) ----
            gated = singles.tile([TR, T], f32r, tag="gated")
            with (
                tc.tile_pool(name="psumA", bufs=1, space="PSUM") as psumA,
                tc.tile_pool(name="wkA", bufs=2) as wkA,
            ):
                pre_ps = psumA.tile([TR, T], f32, tag="pre")
                gin_ps = psumA.tile([TR, T], f32, tag="gin")
                for h in range(2):
                    cols = slice(h * 512, (h + 1) * 512)
                    for k in range(KD):
                        nc.tensor.matmul(pre_ps[:, cols], Wpre_sb[k],
                                         xT_sb[k][:, cols],
                                         start=(k == 0), stop=(k == KD - 1))
                    for k in range(KD):
                        nc.tensor.matmul(gin_ps[:, cols], Wgin_sb[k],
                                         xT_sb[k][:, cols],
                                         start=(k == 0), stop=(k == KD - 1))
                gsigA = wkA.tile([TR, T], f32, tag="gsigA")
                for h in range(2):
                    cols = slice(h * 512, (h + 1) * 512)
                    nc.scalar.activation(gsigA[:, cols], gin_ps[:, cols],
                                         AF.Sigmoid, bias=bgin_sb, scale=1.0)
                nc.vector.scalar_tensor_tensor(
                    out=gated, in0=pre_ps, scalar=bpre_sb, in1=gsigA,
                    op0=op.add, op1=op.mult)

            # ---- fused group loop: scans + gates + mix, 2-deep pipelined --
            with (
                tc.tile_pool(name="psumB", bufs=2, space="PSUM") as psumB,
                tc.tile_pool(name="psumM", bufs=1, space="PSUM") as psumM,
                tc.tile_pool(name="wkB", bufs=3) as wkB,
                tc.tile_pool(name="wkP", bufs=2) as wkP,
                tc.tile_pool(name="wkC", bufs=2) as wkC,
            ):
                zms = [psumM.tile([128, OUT], f32, tag=f"zm{ti}",
                                  name=f"zm{ti}") for ti in range(NTE)]

                def expm_issue(g):
                    """EXPM broadcast matmul for group g + bf16 copy."""
                    zxs = wkB.tile([128, T], sdt, tag="zxs",
                                   name=f"zxs{g}")
                    for h in range(2):
                        cols = slice(h * 512, (h + 1) * 512)
                        zx = psumB.tile([128, 512], f32, tag="px",
                                        name=f"zx{g}_{h}")
                        nc.tensor.matmul(
                            zx,
                            r(EXPM_sb[:, g * 128:(g + 1) * 128]),
                            r(gated[:, cols]), start=True, stop=True)
                        nc.scalar.activation(zxs[:, cols], zx, AF.Copy)
                    return zxs

                zxs_pipe = [expm_issue(0), expm_issue(1)]

                def ln_tail(ti, zm, wk):
                    gsig = gsigs[ti]
                    h_t = wk.tile([128, OUT], f32, tag="h", name="h_t")
                    nc.vector.tensor_tensor(h_t, gsig, zm, op.mult)
                    stats = wk.tile([128, 6], f32, tag="stats", name="stats")
                    nc.vector.bn_stats(stats, h_t)
                    mv = wk.tile([128, 2], f32, tag="mv", name="mv")
                    nc.vector.bn_aggr(mv, stats)
                    sd = wk.tile([128, 1], f32, tag="sd", name="sd")
                    nc.scalar.activation(sd, mv[:, 1:2], AF.Sqrt,
                                         bias=eps_sb, scale=1.0)
                    rstd = wk.tile([128, 1], f32, tag="rstd", name="rstd")
                    nc.vector.reciprocal(rstd, sd)
                    beta = wk.tile([128, 1], f32, tag="beta", name="beta")
                    nc.vector.scalar_tensor_tensor(
                        out=beta, in0=mv[:, 0:1], scalar=-1.0, in1=rstd,
                        op0=op.mult, op1=op.mult)
                    ln = wk.tile([128, OUT], f32, tag="ln", name="ln")
                    nc.scalar.activation(ln, h_t, AF.Identity,
                                         bias=beta, scale=rstd)
                    outt = wk.tile([128, OUT], f32, tag="outt", name="outt")
                    nc.gpsimd.tensor_tensor(outt, ln, sk2s[ti], op.add)
                    outq[ti % 3].dma_start(
                        out=out_d[ti * 128:(ti + 1) * 128, :], in_=outt)

                for g in range(NG):
                    # PE: EXPM for group g+2 (lands ahead of mix(g))
                    if g + 2 < NG:
                        zxs_pipe.append(expm_issue(g + 2))
                    zxs = zxs_pipe[g]
                    # PE+Act: gate/skip for token tile g
                    ti = g
                    tcols = slice(ti * 128, (ti + 1) * 128)
                    gout_ps = psumB.tile([128, OUT], f32, tag="px",
                                         name=f"gout{ti}")
                    for k in range(KD):
                        nc.tensor.matmul(gout_ps, xT_sb[k][:, tcols],
                                         Wgout_sb[k], start=(k == 0),
                                         stop=(not wb and k == KD - 1))
                    if wb:
                        nc.tensor.matmul(gout_ps, r(ones_sb), r(bgout_sb),
                                         start=False, stop=True)
                    nc.scalar.activation(gsigs[ti], gout_ps, AF.Sigmoid)
                    nc.scalar.activation(omgs[ti], gout_ps, AF.Sigmoid,
                                         scale=-1.0)
                    skip_ps = psumB.tile([128, OUT], f32, tag="px",
                                         name=f"skip{ti}")
                    for k in range(KD):
                        nc.tensor.matmul(skip_ps, xT_sb[k][:, tcols],
                                         Wskip_sb[k], start=(k == 0),
                                         stop=(not wb and k == KD - 1))
                    if wb:
                        nc.tensor.matmul(skip_ps, r(ones_sb), r(bskip_sb),
                                         start=False, stop=True)
                    nc.scalar.activation(skips[ti], skip_ps, AF.Copy)

                    # DVE: modulate, scan, demodulate products
                    inA = wkB.tile([128, T], sdt, tag="inA")
                    inB = wkB.tile([128, T], sdt, tag="inB")
                    nc.vector.tensor_tensor(inA, COS_sb, zxs, op.mult)
                    nc.vector.tensor_tensor(inB, SIN_sb, zxs, op.mult)
                    a_t = wkB.tile([128, T], sdt, tag="scnA")
                    b_t = wkB.tile([128, T], sdt, tag="scnB")
                    dec_b = DEC_sb[:, g:g + 1].to_broadcast((128, T))
                    nc.vector.tensor_tensor_scan(
                        a_t, dec_b, inA, 0.0, op.mult, op.add)
                    nc.vector.tensor_tensor_scan(
                        b_t, dec_b, inB, 0.0, op.mult, op.add)
                    p1 = wkP.tile([128, T], sdt, tag="p1")
                    p2 = wkP.tile([128, T], sdt, tag="p2")
                    p3 = wkP.tile([128, T], sdt, tag="p3")
                    p4 = wkP.tile([128, T], sdt, tag="p4")
                    nc.vector.tensor_tensor(p1, COS_sb, a_t, op.mult)
                    nc.vector.tensor_tensor(p2, SIN_sb, b_t, op.mult)
                    nc.vector.tensor_tensor(p3, SIN_sb, a_t, op.mult)
                    nc.vector.tensor_tensor(p4, COS_sb, b_t, op.mult)
                    # Pool: combine into state, then tile-g sk2
                    nc.gpsimd.tensor_tensor(sre[g], p1, p2, op.add)
                    nc.gpsimd.tensor_tensor(sim[g], p3, p4, op.subtract)
                    nc.gpsimd.tensor_tensor(sk2s[ti], omgs[ti], skips[ti],
                                            op.mult)
                    # PE: mix matmuls for group g into the NTE psum tiles
                    for tj in range(NTE):
                        tc2 = slice(tj * 128, (tj + 1) * 128)
                        nc.tensor.matmul(zms[tj], sre[g][:, tc2],
                                         Wmre_sb[g], start=(g == 0),
                                         stop=False, skip_group_check=True)
                        nc.tensor.matmul(zms[tj], sim[g][:, tc2],
                                         Wmim_sb[g], start=False,
                                         stop=(not wb and g == NG - 1),
                                         skip_group_check=True)

                for tj in range(NTE):
                    if wb:
                        nc.tensor.matmul(zms[tj], r(ones_sb), r(bmix_sb),
                                         start=False, stop=True,
                                         skip_group_check=True)
                    ln_tail(tj, zms[tj], wkC)

            # ---- stage C: remaining mix tiles + LN tail ----
            with (
                tc.tile_pool(name="psumC", bufs=2, space="PSUM") as psumC,
                tc.tile_pool(name="wkC2", bufs=2) as wkC2,
            ):
                for ti in range(NTE, NT):
                    tcols = slice(ti * 128, (ti + 1) * 128)
                    zm = psumC.tile([128, OUT], f32, tag="zm", name="zm")
                    for g in range(NG):
                        nc.tensor.matmul(zm, sre[g][:, tcols], Wmre_sb[g],
                                         start=(g == 0), stop=False)
                        nc.tensor.matmul(zm, sim[g][:, tcols], Wmim_sb[g],
                                         start=False,
                                         stop=(not wb and g == NG - 1))
                    if wb:
                        nc.tensor.matmul(zm, r(ones_sb), r(bmix_sb),
                                         start=False, stop=True)
                    ln_tail(ti, zm, wkC2)

    nc.compile()
    return nc


def host_prep(inputs):
    """Compute per-core input maps from the full problem inputs."""
    import ml_dtypes

    sdt_np = ml_dtypes.bfloat16 if STREAM_BF16 else np.float32

    x = np.asarray(inputs["x"], np.float32)
    a = np.abs(np.asarray(inputs["ffa_a"], np.float64))       # [TR]
    b = np.asarray(inputs["ffa_b"], np.float64)               # [CTX]
    t = np.arange(T, dtype=np.float64)

    cos_cols = np.cos(b[:, None] * t[None, :])                # [CTX, T]
    sin_cols = np.sin(b[:, None] * t[None, :])
    COS = np.tile(cos_cols, (8, 1)).astype(sdt_np)            # [128, T]
    SIN = np.tile(sin_cols, (8, 1)).astype(sdt_np)

    dec = np.exp(-a).astype(np.float32)                       # [TR]
    rr = np.arange(128)
    DEC = np.empty((128, NG), np.float32)
    for g in range(NG):
        DEC[:, g] = dec[8 * g + rr // 16]

    col = np.arange(NCH)
    EXPM = (np.arange(TR)[:, None] == (col[None, :] // CTX)).astype(
        np.float32)

    Wm = np.asarray(inputs["W_mix"], np.float32).reshape(TR, 2, CTX, OUT)
    Wmre = np.ascontiguousarray(Wm[:, 0].reshape(NCH, OUT)).astype(sdt_np)
    Wmim = np.ascontiguousarray(Wm[:, 1].reshape(NCH, OUT)).astype(sdt_np)

    shared = {
        "Wpre": np.ascontiguousarray(inputs["W_pre"], np.float32).astype(sdt_np),
        "Wgin": np.ascontiguousarray(inputs["W_gin"], np.float32).astype(sdt_np),
        "Wgout": np.ascontiguousarray(inputs["W_gout"], np.float32).astype(sdt_np),
        "Wskip": np.ascontiguousarray(inputs["W_skip"], np.float32).astype(sdt_np),
        "Wmre": Wmre, "Wmim": Wmim,
        "COS": COS, "SIN": SIN, "DEC": DEC, "EXPM": EXPM,
        "bpre": np.asarray(inputs["b_pre"], np.float32).reshape(TR, 1),
        "bgin": np.asarray(inputs["b_gin"], np.float32).reshape(TR, 1),
        "bgout": np.asarray(inputs["b_gout"], np.float32).reshape(1, OUT),
        "bskip": np.asarray(inputs["b_skip"], np.float32).reshape(1, OUT),
        "bmix": np.asarray(inputs["b_mix"], np.float32).reshape(1, OUT),
        "ones": np.ones((1, 128), np.float32),
    }
    in_maps = []
    for core in range(B):
        m = dict(shared)
        m["xT"] = np.ascontiguousarray(x[core].T).astype(sdt_np)
        in_maps.append(m)
    return in_maps


def kernel(**inputs):
    from concourse import bass_utils

    wb = any(
        np.any(np.asarray(inputs[k]))
        for k in ("b_pre", "b_gin", "b_gout", "b_skip", "b_mix")
    )
    key = f"nc_wb{wb}"
    if key not in _cache:
        _cache[key] = build_program(with_bias=wb)
    nc = _cache[key]
    in_maps = host_prep(inputs)
    res = bass_utils.run_bass_kernel_spmd(nc, in_maps, core_ids=list(range(B)))
    return np.stack([res.results[i]["out"] for i in range(B)])


# revision 16
# speedup vs baseline: 1.3160x; 1.3160x over previous
"""FFM cell kernel for Trainium2, 8 NeuronCores, batch-parallel.

Math: per batch element b,
    gated[t,m] = (x@W_pre + b_pre)[t,m] * sigmoid(x@W_gin + b_gin)[t,m]
    state[t,m,c] = sum_{s<=t} exp((a_m + i*b_c)*(t-s)) * gated[s,m]
    zm = [state.re, state.im] @ W_mix + b_mix
    out = LN(zm * sig(gout)) + skip * (1 - sig(gout))

The complex diagonal recurrence is decoupled into two *real* first-order
scans using angle addition (z is real):
    A[t,ch] = e^{a_m} A[t-1,ch] + cos(b_c t) z[t,m]
    B[t,ch] = e^{a_m} B[t-1,ch] + sin(b_c t) z[t,m]
    state_re = cos(b_c t) A + sin(b_c t) B
    state_im = sin(b_c t) A - cos(b_c t) B
mapped onto the DVE hardware scan (tensor_tensor_scan), channels (m,c) on
partitions, time on the free dim; cos/sin tables host-precomputed.

Engine split (latency-oriented):
  PE   : all matmuls, 2-deep software-pipelined so group g+2's broadcast
         (EXPM) matmul is issued before group g's mix matmuls.
  DVE  : modulation TTs, the two scans, the four demod products, LN stats.
  Pool : demod add/sub (sre/sim), sk2 = (1-gate)*skip, final out add.
  Act  : PSUM->SBUF bf16 copies, sigmoids (gate and 1-gate directly from
         PSUM via scale=-1), LN normalize, rsqrt.
Matmuls run as float32r/bf16. Sharding: batch element -> core; everything
replicated; no collectives.
"""

import numpy as np

B, T, D = 8, 1024, 512
TR, CTX, OUT = 64, 16, 512
EPS = 1e-6
NCH = TR * CTX   # 1024 scan channels
NG = NCH // 128  # 8 channel groups of 128 partitions
NT = T // 128    # 8 token tiles
KD = D // 128    # 4 contraction chunks over D
NTE = 6          # token tiles whose zm accumulates in PSUM during the loop

STREAM_BF16 = True   # bf16 modulation/post stream (DVE 2x) vs fp32

_cache = {}


def build_program(n_rep=1, with_bias=True):
    """Build + compile the Bass program (single NEFF, SPMD on 8 cores).

    n_rep > 1 repeats the whole pipeline (incl. DMA loads) for
    differential wall-clock timing; each repeat rewrites the output."""
    import concourse.bacc as bacc
    import concourse.tile as tile
    import concourse.mybir as mybir
    from concourse.alu_op_type import AluOpType as op

    f32 = mybir.dt.float32
    f32r = mybir.dt.float32r
    bf16 = mybir.dt.bfloat16
    sdt = bf16 if STREAM_BF16 else f32
    AF = mybir.ActivationFunctionType

    def r(ap):  # fp32 -> fp32r view for fast PE matmul
        return ap.bitcast(f32r) if ap.dtype == f32 else ap

    wb = with_bias
    nc = bacc.Bacc("TRN2", target_bir_lowering=False, debug=False)

    def din(name, shape, dt=f32):
        return nc.dram_tensor(name, shape, dt, kind="ExternalInput").ap()

    # packed inputs: few big DMAs beat many small ones (per-DMA overhead)
    xTp = din("xTp", (128, KD * T), sdt)        # x chunks along free dim
    Wab = din("Wab", (128, 2 * KD * TR), sdt)   # Wpre | Wgin chunks
    CS = din("CS", (128, 2 * T), sdt)           # COS | SIN
    Wgs = din("Wgs", (128, 2 * KD * OUT), sdt)  # Wgout | Wskip chunks
    Wm = din("Wm", (128, 2 * NG * OUT), sdt)    # Wmre g0..7 | Wmim g0..7
    DEC = din("DEC", (128, NG))           # col g: exp(-|a_{8g + rr//16}|)
    EXPM = din("EXPM", (TR, NCH), f32r)   # 0/1: EXPM[m, col] = (m == col//16)
    if wb:
        bpre = din("bpre", (TR, 1))
        bgin = din("bgin", (TR, 1))
        bgout = din("bgout", (1, OUT), f32r)
        bskip = din("bskip", (1, OUT), f32r)
        bmix = din("bmix", (1, OUT), f32r)
        ones = din("ones", (1, 128), f32r)
    out_d = nc.dram_tensor("out", (T, OUT), f32, kind="ExternalOutput").ap()

    outq = [nc.sync, nc.scalar]

    with tile.TileContext(nc) as tc:
      for _rep in range(n_rep):
        with (
            tc.tile_pool(name="singles", bufs=1) as singles,
            tc.tile_pool(name="states", bufs=1) as states,
        ):
            def load(ap_dram, shape, tag, dt=f32, q=nc.sync):
                t = singles.tile(shape, dt, tag=tag, name=tag)
                q.dma_start(out=t, in_=ap_dram)
                return t

            xTp_sb = load(xTp, [128, KD * T], "xTp", sdt)
            xT_sb = [xTp_sb[:, k * T:(k + 1) * T] for k in range(KD)]
            Wab_sb = load(Wab, [128, 2 * KD * TR], "Wab", sdt)
            WPG_sb = [Wab_sb[:, k * 2 * TR:(k + 1) * 2 * TR]
                      for k in range(KD)]
            EXPM_sb = load(EXPM, [TR, NCH], "expm", f32r)
            CS_sb = load(CS, [128, 2 * T], "cs", sdt)
            COS_sb = CS_sb[:, 0:T]
            SIN_sb = CS_sb[:, T:2 * T]
            DEC_sb = load(DEC, [128, NG], "dec")
            Wgs_sb = load(Wgs, [128, 2 * KD * OUT], "wgs", sdt)
            Wgout_sb = [Wgs_sb[:, k * OUT:(k + 1) * OUT] for k in range(KD)]
            Wskip_sb = [Wgs_sb[:, (KD + k) * OUT:(KD + k + 1) * OUT]
                        for k in range(KD)]
            Wm_sb = load(Wm, [128, 2 * NG * OUT], "wm", sdt)
            Wmre_sb = [Wm_sb[:, g * OUT:(g + 1) * OUT] for g in range(NG)]
            Wmim_sb = [Wm_sb[:, (NG + g) * OUT:(NG + g + 1) * OUT]
                       for g in range(NG)]
            if wb:
                bpre_sb = load(bpre, [TR, 1], "bpre")
                bgin_sb = load(bgin, [TR, 1], "bgin")
                bgout_sb = load(bgout, [1, OUT], "bgout", f32r)
                bskip_sb = load(bskip, [1, OUT], "bskip", f32r)
                bmix_sb = load(bmix, [1, OUT], "bmix", f32r)
                ones_sb = load(ones, [1, 128], "ones", f32r)
            else:
                bpre_sb = singles.tile([TR, 1], f32, tag="bpre")
                nc.vector.memset(bpre_sb, 0.0)
                bgin_sb = bpre_sb
            eps_sb = singles.tile([128, 1], f32, tag="eps")
            nc.vector.memset(eps_sb, EPS)

            sre = [states.tile([128, T], sdt, tag=f"sre{g}", name=f"sre{g}")
                   for g in range(NG)]
            sim = [states.tile([128, T], sdt, tag=f"sim{g}", name=f"sim{g}")
                   for g in range(NG)]
            gsigs = [states.tile([128, OUT], f32, tag=f"gsig{ti}",
                                 name=f"gsig{ti}") for ti in range(NT)]
            skips = [states.tile([128, OUT], f32, tag=f"skip{ti}",
                                 name=f"skip{ti}") for ti in range(NT)]
            omgs = [states.tile([128, OUT], f32, tag=f"omg{ti}",
                                name=f"omg{ti}") for ti in range(NT)]
            sk2s = [states.tile([128, OUT], f32, tag=f"sk2{ti}",
                                name=f"sk2{ti}") for ti in range(NT)]

            # ---- stage A: gated = (pre + bpre) * sig(gin + bgin) ----
            gated = singles.tile([TR, T], f32r, tag="gated")
            with (
                tc.tile_pool(name="psumA", bufs=1, space="PSUM") as psumA,
                tc.tile_pool(name="wkA", bufs=2) as wkA,
            ):
                # fused: out rows 0:TR = pre, TR:2TR = gin (full 128-wide PE)
                pg_ps = psumA.tile([2 * TR, T], f32, tag="pg")
                for h in range(2):
                    cols = slice(h * 512, (h + 1) * 512)
                    for k in range(KD):
                        nc.tensor.matmul(pg_ps[:, cols], WPG_sb[k],
                                         xT_sb[k][:, cols],
                                         start=(k == 0), stop=(k == KD - 1))
                gsigA = wkA.tile([TR, T], f32, tag="gsigA")
                for h in range(2):
                    cols = slice(h * 512, (h + 1) * 512)
                    nc.scalar.activation(gsigA[:, cols],
                                         pg_ps[TR:2 * TR, cols],
                                         AF.Sigmoid, bias=bgin_sb, scale=1.0)
                    nc.vector.scalar_tensor_tensor(
                        out=gated[:, cols], in0=pg_ps[0:TR, cols],
                        scalar=bpre_sb, in1=gsigA[:, cols],
                        op0=op.add, op1=op.mult)

            # ---- fused group loop: scans + gates + mix, 2-deep pipelined --
            with (
                tc.tile_pool(name="psumB", bufs=2, space="PSUM") as psumB,
                tc.tile_pool(name="psumM", bufs=1, space="PSUM") as psumM,
                tc.tile_pool(name="wkB", bufs=3) as wkB,
                tc.tile_pool(name="wkP", bufs=2) as wkP,
                tc.tile_pool(name="wkC", bufs=2) as wkC,
            ):
                zms = [psumM.tile([128, OUT], f32, tag=f"zm{ti}",
                                  name=f"zm{ti}") for ti in range(NTE)]

                def expm_issue(g):
                    """EXPM broadcast matmul for group g + bf16 copy."""
                    zxs = wkB.tile([128, T], sdt, tag="zxs",
                                   name=f"zxs{g}")
                    for h in range(2):
                        cols = slice(h * 512, (h + 1) * 512)
                        zx = psumB.tile([128, 512], f32, tag="px",
                                        name=f"zx{g}_{h}")
                        nc.tensor.matmul(
                            zx,
                            r(EXPM_sb[:, g * 128:(g + 1) * 128]),
                            r(gated[:, cols]), start=True, stop=True)
                        nc.scalar.activation(zxs[:, cols], zx, AF.Copy)
                    return zxs

                zxs_pipe = [expm_issue(0), expm_issue(1)]

                def ln_tail(ti, zm, wk):
                    gsig = gsigs[ti]
                    h_t = wk.tile([128, OUT], f32, tag="h", name="h_t")
                    nc.vector.tensor_tensor(h_t, gsig, zm, op.mult)
                    stats = wk.tile([128, 6], f32, tag="stats", name="stats")
                    nc.vector.bn_stats(stats, h_t)
                    mv = wk.tile([128, 2], f32, tag="mv", name="mv")
                    nc.vector.bn_aggr(mv, stats)
                    sd = wk.tile([128, 1], f32, tag="sd", name="sd")
                    nc.scalar.activation(sd, mv[:, 1:2], AF.Sqrt,
                                         bias=eps_sb, scale=1.0)
                    rstd = wk.tile([128, 1], f32, tag="rstd", name="rstd")
                    nc.vector.reciprocal(rstd, sd)
                    beta = wk.tile([128, 1], f32, tag="beta", name="beta")
                    nc.vector.scalar_tensor_tensor(
                        out=beta, in0=mv[:, 0:1], scalar=-1.0, in1=rstd,
                        op0=op.mult, op1=op.mult)
                    ln = wk.tile([128, OUT], f32, tag="ln", name="ln")
                    nc.scalar.activation(ln, h_t, AF.Identity,
                                         bias=beta, scale=rstd)
                    outt = wk.tile([128, OUT], f32, tag="outt", name="outt")
                    oeng = nc.vector if ti >= 5 else nc.gpsimd
                    oeng.tensor_tensor(outt, ln, sk2s[ti], op.add)
                    outq[ti % 2].dma_start(
                        out=out_d[ti * 128:(ti + 1) * 128, :], in_=outt)

                for g in range(NG):
                    # PE: EXPM for group g+2 (lands ahead of mix(g))
                    if g + 2 < NG:
                        zxs_pipe.append(expm_issue(g + 2))
                    zxs = zxs_pipe[g]
                    # PE+Act: gate/skip, shifted early (iter 0: tiles 0+1)
                    gs_tiles = [0, 1] if g == 0 else (
                        [g + 1] if g + 1 < NT else [])
                    for ti in gs_tiles:
                        tcols = slice(ti * 128, (ti + 1) * 128)
                        gout_ps = psumB.tile([128, OUT], f32, tag="px",
                                             name=f"gout{ti}")
                        for k in range(KD):
                            nc.tensor.matmul(gout_ps, xT_sb[k][:, tcols],
                                             Wgout_sb[k], start=(k == 0),
                                             stop=(not wb and k == KD - 1))
                        if wb:
                            nc.tensor.matmul(gout_ps, r(ones_sb),
                                             r(bgout_sb),
                                             start=False, stop=True)
                        nc.scalar.activation(gsigs[ti], gout_ps, AF.Sigmoid)
                        nc.scalar.activation(omgs[ti], gout_ps, AF.Sigmoid,
                                             scale=-1.0)
                        skip_ps = psumB.tile([128, OUT], f32, tag="px",
                                             name=f"skip{ti}")
                        for k in range(KD):
                            nc.tensor.matmul(skip_ps, xT_sb[k][:, tcols],
                                             Wskip_sb[k], start=(k == 0),
                                             stop=(not wb and k == KD - 1))
                        if wb:
                            nc.tensor.matmul(skip_ps, r(ones_sb),
                                             r(bskip_sb),
                                             start=False, stop=True)
                        nc.scalar.activation(skips[ti], skip_ps, AF.Copy)
                    ti = g

                    # DVE: modulate, scan, demodulate products
                    inA = wkB.tile([128, T], sdt, tag="inA")
                    inB = wkB.tile([128, T], sdt, tag="inB")
                    nc.vector.tensor_tensor(inA, COS_sb, zxs, op.mult)
                    nc.vector.tensor_tensor(inB, SIN_sb, zxs, op.mult)
                    a_t = wkB.tile([128, T], sdt, tag="scnA")
                    b_t = wkB.tile([128, T], sdt, tag="scnB")
                    dec_b = DEC_sb[:, g:g + 1].to_broadcast((128, T))
                    nc.vector.tensor_tensor_scan(
                        a_t, dec_b, inA, 0.0, op.mult, op.add)
                    nc.vector.tensor_tensor_scan(
                        b_t, dec_b, inB, 0.0, op.mult, op.add)
                    p1 = wkP.tile([128, T], sdt, tag="p1")
                    p2 = wkP.tile([128, T], sdt, tag="p2")
                    p3 = wkP.tile([128, T], sdt, tag="p3")
                    p4 = wkP.tile([128, T], sdt, tag="p4")
                    last = g == NG - 1
                    cmb = nc.vector if last else nc.gpsimd
                    nc.vector.tensor_tensor(p1, COS_sb, a_t, op.mult)
                    nc.vector.tensor_tensor(p2, SIN_sb, b_t, op.mult)
                    cmb.tensor_tensor(sre[g], p1, p2, op.add)
                    nc.vector.tensor_tensor(p3, SIN_sb, a_t, op.mult)
                    nc.vector.tensor_tensor(p4, COS_sb, b_t, op.mult)
                    cmb.tensor_tensor(sim[g], p3, p4, op.subtract)
                    nc.gpsimd.tensor_tensor(sk2s[ti], omgs[ti], skips[ti],
                                            op.mult)
                    if g == NG - 1:
                        dummy = wkB.tile([128, 1], f32, tag="dmy")
                        nc.scalar.activation(dummy, eps_sb, AF.Sqrt)
                    # PE: mix matmuls for group g into the NTE psum tiles
                    for tj in range(NTE):
                        tc2 = slice(tj * 128, (tj + 1) * 128)
                        nc.tensor.matmul(zms[tj], sre[g][:, tc2],
                                         Wmre_sb[g], start=(g == 0),
                                         stop=False, skip_group_check=True)
                        nc.tensor.matmul(zms[tj], sim[g][:, tc2],
                                         Wmim_sb[g], start=False,
                                         stop=(not wb and g == NG - 1),
                                         skip_group_check=True)

                zmC = {}
                for ti in range(NTE, NT):
                    tcols = slice(ti * 128, (ti + 1) * 128)
                    zm = psumB.tile([128, OUT], f32, tag="px",
                                    name=f"zmC{ti}")
                    for g in range(NG):
                        nc.tensor.matmul(zm, sre[g][:, tcols], Wmre_sb[g],
                                         start=(g == 0), stop=False)
                        nc.tensor.matmul(zm, sim[g][:, tcols], Wmim_sb[g],
                                         start=False,
                                         stop=(not wb and g == NG - 1))
                    if wb:
                        nc.tensor.matmul(zm, r(ones_sb), r(bmix_sb),
                                         start=False, stop=True)
                    zmC[ti] = zm
                for tj in range(NTE):
                    if wb:
                        nc.tensor.matmul(zms[tj], r(ones_sb), r(bmix_sb),
                                         start=False, stop=True,
                                         skip_group_check=True)
                    ln_tail(tj, zms[tj], wkC)
                for ti in range(NTE, NT):
                    ln_tail(ti, zmC[ti], wkC)

    nc.compile()
    return nc


def host_prep(inputs, wb):
    """Compute per-core input maps from the full problem inputs."""
    import ml_dtypes

    sdt_np = ml_dtypes.bfloat16 if STREAM_BF16 else np.float32

    x = np.asarray(inputs["x"], np.float32)
    a = np.abs(np.asarray(inputs["ffa_a"], np.float64))       # [TR]
    b = np.asarray(inputs["ffa_b"], np.float64)               # [CTX]
    t = np.arange(T, dtype=np.float64)

    cos_cols = np.cos(b[:, None] * t[None, :])                # [CTX, T]
    sin_cols = np.sin(b[:, None] * t[None, :])
    COS = np.tile(cos_cols, (8, 1)).astype(sdt_np)            # [128, T]
    SIN = np.tile(sin_cols, (8, 1)).astype(sdt_np)
    COS = np.ascontiguousarray(COS); SIN = np.ascontiguousarray(SIN)

    dec = np.exp(-a).astype(np.float32)                       # [TR]
    rr = np.arange(128)
    DEC = np.empty((128, NG), np.float32)
    for g in range(NG):
        DEC[:, g] = dec[8 * g + rr // 16]

    col = np.arange(NCH)
    EXPM = (np.arange(TR)[:, None] == (col[None, :] // CTX)).astype(
        np.float32)

    Wm = np.asarray(inputs["W_mix"], np.float32).reshape(TR, 2, CTX, OUT)
    Wmre = np.ascontiguousarray(Wm[:, 0].reshape(NCH, OUT))
    Wmim = np.ascontiguousarray(Wm[:, 1].reshape(NCH, OUT))

    def chunks(w):  # (D, N) -> [128, KD*N] stacking 128-row chunks
        w = np.asarray(w, np.float32)
        n = w.shape[1]
        out = np.empty((128, KD * n), np.float32)
        for k in range(KD):
            out[:, k * n:(k + 1) * n] = w[k * 128:(k + 1) * 128, :]
        return out

    wp, wg = chunks(inputs["W_pre"]), chunks(inputs["W_gin"])
    Wab = np.empty((128, 2 * KD * TR), np.float32)
    for k in range(KD):
        Wab[:, k * 2 * TR:k * 2 * TR + TR] = wp[:, k * TR:(k + 1) * TR]
        Wab[:, k * 2 * TR + TR:(k + 1) * 2 * TR] = wg[:, k * TR:(k + 1) * TR]
    Wab = Wab.astype(sdt_np)
    Wgs = np.concatenate([chunks(inputs["W_gout"]),
                          chunks(inputs["W_skip"])], axis=1).astype(sdt_np)
    Wmp = np.empty((128, 2 * NG * OUT), np.float32)
    for g in range(NG):
        Wmp[:, g * OUT:(g + 1) * OUT] = Wmre[g * 128:(g + 1) * 128, :]
        Wmp[:, (NG + g) * OUT:(NG + g + 1) * OUT] = \
            Wmim[g * 128:(g + 1) * 128, :]
    shared = {
        "Wab": Wab, "Wgs": Wgs, "Wm": Wmp.astype(sdt_np),
        "CS": np.concatenate([COS, SIN], axis=1),
        "DEC": DEC, "EXPM": EXPM,
    }
    if wb:
        shared.update({
            "bpre": np.asarray(inputs["b_pre"], np.float32).reshape(TR, 1),
            "bgin": np.asarray(inputs["b_gin"], np.float32).reshape(TR, 1),
            "bgout": np.asarray(inputs["b_gout"],
                                np.float32).reshape(1, OUT),
            "bskip": np.asarray(inputs["b_skip"],
                                np.float32).reshape(1, OUT),
            "bmix": np.asarray(inputs["b_mix"], np.float32).reshape(1, OUT),
            "ones": np.ones((1, 128), np.float32),
        })
    in_maps = []
    for core in range(B):
        m = dict(shared)
        xc = np.ascontiguousarray(x[core].T).astype(sdt_np)
        xp = np.empty((128, KD * T), sdt_np)
        for k in range(KD):
            xp[:, k * T:(k + 1) * T] = xc[k * 128:(k + 1) * 128, :]
        m["xTp"] = xp
        in_maps.append(m)
    return in_maps


def kernel(**inputs):
    from concourse import bass_utils

    wb = any(
        np.any(np.asarray(inputs[k]))
        for k in ("b_pre", "b_gin", "b_gout", "b_skip", "b_mix")
    )
    key = f"nc_wb{wb}"
    if key not in _cache:
        _cache[key] = build_program(with_bias=wb)
    nc = _cache[key]
    in_maps = host_prep(inputs, wb)
    res = bass_utils.run_bass_kernel_spmd(nc, in_maps, core_ids=list(range(B)))
    return np.stack([res.results[i]["out"] for i in range(B)])
